# revision 36
# baseline (speedup 1.0000x reference)
"""Trainium2 Bass kernel for nn_MinimaxKnnHeadFast (segmented top-k KL head).

Strategy (8 cores, SPMD, no collectives):
  * Host groups candidates by segment and packs segments into 200 blocks
    (<=128 segments and <=BCAP candidates per block); 25 blocks per core.
  * Device phase A: per 128-candidate tile: dense-load student rows,
    indirect-gather teacher rows, compute per-row KL(dist), indirect-scatter
    dist into the block's [128 seg x K slot] DRAM buffer.
  * Device phase B: per block: load dist tile, mask pad slots to -inf,
    vector.max / max_index -> r-th largest value + index per segment
    (tie semantics identical to the reference's iterative scatter_max).
  * Device phase C: chained indirect gather of the selected teacher rows.
  * Host maps slots back to original ids and patches invalid segments.
"""

import numpy as np

B = 200000
S = 25000
E = 50000
A = 8
C = 128
TEMP = 2.0
NCORES = 8

SEGBLK = 128          # segments per block (= SBUF partitions)
K = 32                # dist slots per segment (max candidates/segment)
NT = 9                # 128-candidate tiles per block
BCAP = NT * 128       # candidate slots per block (1152)
NBLK = 25             # blocks per core
NSEG = NBLK * SEGBLK  # segment slots per core (3200)
NCAND = NBLK * BCAP   # candidate slots per core (28800)
NBINS = NCORES * NBLK  # 200 blocks total
TR = E * A            # teacher table rows (flattened [E*A, C])


def build_program(r, nblk=NBLK, nt=NT, k=K, tr=TR, c=C):
    """Build the per-core Bass program (same on all cores; data differs).

    Wait-limit discipline: walrus allows ~1 sync-wait per instruction, so
    every DMA-produced tile is "pre-touched" by a tiny op on each consuming
    engine before any multi-input op, and phase B/C tiles use no-reuse pools
    (bufs=nblk) so no slot-release waits exist.
    """
    import concourse.bass as bass
    import concourse.bacc as bacc
    import concourse.mybir as mybir
    from concourse.tile import TileContext

    # Force Exp and Ln to resolve to the one act-table set that holds both
    # ("natural_log_exp_and_others"), so the kernel loads the table once
    # instead of swapping Exp<->Ln sets every tile (~1.3us per swap).
    if not getattr(bacc, "_act_tables_patched", False):
        _orig_gat = bacc.get_activation_tables

        def _patched_gat(arch):
            t = dict(_orig_gat(arch))
            Exp = mybir.ActivationFunctionType.Exp
            Ln = mybir.ActivationFunctionType.Ln
            keep = "natural_log_exp_and_others"
            if keep in t and Exp in t[keep] and Ln in t[keep]:
                for name in t:
                    if name != keep:
                        t[name] = t[name] - {Exp, Ln}
            return t

        bacc.get_activation_tables = _patched_gat
        bacc._act_tables_patched = True

    f32 = mybir.dt.float32
    i32 = mybir.dt.int32
    u32 = mybir.dt.uint32
    X = mybir.AxisListType.X
    Alu = mybir.AluOpType
    Act = mybir.ActivationFunctionType
    IOA = bass.IndirectOffsetOnAxis

    bcap = nt * 128
    nseg = nblk * SEGBLK
    ncand = nblk * bcap
    na = 2 * nt + 2  # aux cols: trow[0:nt], spos[nt:2nt], cnt, soff

    nc = bacc.Bacc(None, target_bir_lowering=False, debug=False)
    stu = nc.dram_tensor("stu", [ncand, c], f32, kind="ExternalInput")
    aux = nc.dram_tensor("aux", [nseg, na], i32, kind="ExternalInput")
    trowf = nc.dram_tensor("trowf", [ncand, 1], i32, kind="ExternalInput")
    teach = nc.dram_tensor("teach", [tr, c], f32, kind="ExternalInput")
    m4o = nc.dram_tensor("m4o", [nseg, 8], f32, kind="ExternalOutput")
    selo = nc.dram_tensor("selo", [nseg, 1], i32, kind="ExternalOutput")
    teao = nc.dram_tensor("teao", [nseg, c], f32, kind="ExternalOutput")
    # +128 dump rows at the tail: pad candidates scatter there harmlessly
    dpads = [nc.dram_tensor(f"dpad{b}", [SEGBLK * k + 128, 1], f32)
             for b in range(nblk)]

    with (
        TileContext(nc) as tc,
        tc.tile_pool(name="big", bufs=3) as bigp,
        tc.tile_pool(name="blk", bufs=3) as blkp,
        tc.tile_pool(name="small", bufs=12) as smp,
        tc.tile_pool(name="nor", bufs=nblk) as norp,   # no-reuse (1 buf/block)
        tc.tile_pool(name="const", bufs=1) as cstp,
    ):
        iota_t = cstp.tile([128, k], i32)
        nc.gpsimd.iota(iota_t[:], pattern=[[1, k]], base=0, channel_multiplier=0)
        ninf_t = cstp.tile([128, k], f32)
        nc.vector.memset(ninf_t[:], -3.0e38)

        for b in range(nblk):
            stu_blk = blkp.tile([128, nt * c], f32, tag="stu_blk")
            nc.sync.dma_start(
                out=stu_blk[:].rearrange("p (t c) -> p t c", t=nt),
                in_=stu[b * bcap:(b + 1) * bcap, :].rearrange(
                    "(t p) c -> p t c", p=128
                ),
            )
            aux_blk = blkp.tile([128, na], i32, tag="aux_blk")
            nc.sync.dma_start(out=aux_blk[:], in_=aux[b * 128:(b + 1) * 128, :])
            trow_cols = aux_blk[:, 0:nt]
            spos_cols = aux_blk[:, nt:2 * nt]
            cnt_col = aux_blk[:, 2 * nt:2 * nt + 1]
            soff_col = aux_blk[:, 2 * nt + 1:2 * nt + 2]

            # per-tile gathers ([128,1] offsets: HW-verified pairing)
            tea_blk = blkp.tile([128, nt * c], f32, tag="tea_blk")
            for t in range(nt):
                nc.gpsimd.indirect_dma_start(
                    out=tea_blk[:, t * c:(t + 1) * c],
                    out_offset=None,
                    in_=teach[:],
                    in_offset=IOA(ap=trow_cols[:, t:t + 1], axis=0),
                )

            d1b = smp.tile([128, nt], f32, tag="d1b")
            ssb = smp.tile([128, nt], f32, tag="ssb")
            stb = smp.tile([128, nt], f32, tag="stb")
            for t in range(nt):
                stu_t = stu_blk[:, t * c:(t + 1) * c]
                tea_t = tea_blk[:, t * c:(t + 1) * c]
                et = bigp.tile([128, c], f32, tag="et")
                nc.scalar.activation(
                    out=et[:], in_=tea_t, func=Act.Exp, scale=0.5,
                    accum_out=stb[:, t:t + 1]
                )
                esd = bigp.tile([128, c], f32, tag="esd")
                nc.scalar.activation(
                    out=esd[:], in_=stu_t, func=Act.Exp, scale=0.5,
                    accum_out=ssb[:, t:t + 1]
                )
                dd = bigp.tile([128, c], f32, tag="dd")
                nc.vector.tensor_sub(dd[:], tea_t, stu_t)
                nc.vector.tensor_mul(dd[:], dd[:], et[:])
                nc.vector.reduce_sum(d1b[:, t:t + 1], dd[:], axis=X)

            # block-level dist: 0.5*d1/st + ln(ss/st), one Ln per block
            rsb = smp.tile([128, nt], f32, tag="rsb")
            nc.vector.reciprocal(rsb[:], stb[:])
            qb = smp.tile([128, nt], f32, tag="qb")
            nc.vector.tensor_mul(qb[:], ssb[:], rsb[:])
            lgb = smp.tile([128, nt], f32, tag="lgb")
            nc.scalar.activation(out=lgb[:], in_=qb[:], func=Act.Ln)
            t1b = smp.tile([128, nt], f32, tag="t1b")
            nc.vector.tensor_mul(t1b[:], d1b[:], rsb[:])
            nc.vector.tensor_scalar_mul(t1b[:], t1b[:], 0.5)
            distb = smp.tile([128, nt], f32, tag="distb")
            nc.vector.tensor_tensor(
                out=distb[:], in0=t1b[:], in1=lgb[:], op=Alu.add)
            # per-column scatters ([128,1] offsets: HW-verified pairing)
            for t in range(nt):
                nc.gpsimd.indirect_dma_start(
                    out=dpads[b][:],
                    out_offset=IOA(ap=spos_cols[:, t:t + 1], axis=0),
                    in_=distb[:, t:t + 1],
                    in_offset=None,
                )

            # ---- phase B: segmented top-r on the block ----
            v = norp.tile([128, k], f32, tag="v")
            nc.sync.dma_start(
                out=v[:],
                in_=dpads[b][:SEGBLK * k, :].rearrange("(p k) x -> p (k x)", p=128),
            )
            padm = norp.tile([128, k], i32, tag="padm")
            nc.vector.tensor_tensor(
                out=padm[:],
                in0=iota_t[:],
                in1=cnt_col.to_broadcast([128, k]),
                op=Alu.is_ge,
            )
            nc.vector.copy_predicated(v[:], padm[:], ninf_t[:])
            mx = norp.tile([128, 8], f32, tag="mx")
            nc.vector.max(mx[:], v[:])
            mi = norp.tile([128, 8], u32, tag="mi")
            nc.vector.max_index(mi[:], mx[:], v[:])
            mi32 = norp.tile([128, 8], i32, tag="mi32")
            nc.vector.tensor_copy(mi32[:], mi[:])
            selp = norp.tile([128, 1], i32, tag="selp")
            nc.vector.tensor_tensor(
                out=selp[:], in0=mi32[:, r - 1:r], in1=soff_col, op=Alu.add
            )
            nc.vector.tensor_scalar(selp[:], selp[:], 0, None, op0=Alu.max)
            nc.vector.tensor_scalar(selp[:], selp[:], ncand - 1, None, op0=Alu.min)
            nc.sync.dma_start(out=m4o[b * 128:(b + 1) * 128, :], in_=mx[:])
            nc.sync.dma_start(out=selo[b * 128:(b + 1) * 128, :], in_=selp[:])

            # ---- phase C: gather selected teacher rows ----
            rsel = norp.tile([128, 1], i32, tag="rsel")
            nc.gpsimd.indirect_dma_start(
                out=rsel[:],
                out_offset=None,
                in_=trowf[:],
                in_offset=IOA(ap=selp[:, :1], axis=0),
            )
            nc.vector.tensor_scalar(rsel[:], rsel[:], 0, None, op0=Alu.max)
            nc.vector.tensor_scalar(rsel[:], rsel[:], tr - 1, None, op0=Alu.min)
            teas = norp.tile([128, c], f32, tag="teas")
            nc.gpsimd.indirect_dma_start(
                out=teas[:],
                out_offset=None,
                in_=teach[:],
                in_offset=IOA(ap=rsel[:, :1], axis=0),
            )
            nc.sync.dma_start(out=teao[b * 128:(b + 1) * 128, :], in_=teas[:])

    nc.compile()
    return nc


def prepare_inputs(nn_mask, example_indices, stu_logits, augmented_indices):
    """Host-side packing. Returns (in_maps, seg_dev, seg_row, orig_map)."""
    nn = np.asarray(nn_mask, dtype=np.int64)
    ex = np.asarray(example_indices, dtype=np.int64)
    aug = np.asarray(augmented_indices, dtype=np.int64)
    stu_np = np.asarray(stu_logits, dtype=np.float32)

    counts = np.bincount(nn, minlength=S)
    assert counts.max() <= K, f"segment with {counts.max()} > {K} candidates"

    # round-robin (by descending count) assignment of segments to bins
    seg_sorted = np.argsort(-counts, kind="stable")
    bin_of_seg = np.empty(S, dtype=np.int64)
    bin_of_seg[seg_sorted] = np.arange(S) % NBINS
    bin_cand = np.bincount(bin_of_seg, weights=counts, minlength=NBINS).astype(np.int64)
    assert bin_cand.max() <= BCAP, f"bin with {bin_cand.max()} > {BCAP} candidates"
    nseg_per_bin = np.bincount(bin_of_seg, minlength=NBINS)
    assert nseg_per_bin.max() <= SEGBLK

    # per-segment rank within its bin (order by segment id)
    sord = np.lexsort((np.arange(S), bin_of_seg))
    bin_seg_start = np.concatenate(([0], np.cumsum(nseg_per_bin)))
    seg_rank = np.empty(S, dtype=np.int64)
    seg_rank[sord] = np.arange(S) - bin_seg_start[bin_of_seg[sord]]

    # order candidates by (bin, segment id, original index)
    bin_c = bin_of_seg[nn]
    ordc = np.lexsort((np.arange(B), nn, bin_c))
    bin_c_sorted = bin_c[ordc]
    nn_sorted = nn[ordc]
    bin_cand_start = np.concatenate(([0], np.cumsum(np.bincount(bin_c, minlength=NBINS))))
    pos_in_bin = np.arange(B) - bin_cand_start[bin_c_sorted]

    # position within segment run (j slot in the K-wide dist row)
    newseg = np.empty(B, dtype=bool)
    newseg[0] = True
    newseg[1:] = nn_sorted[1:] != nn_sorted[:-1]
    runstart = np.flatnonzero(newseg)
    runid = np.cumsum(newseg) - 1
    j_in_seg = np.arange(B) - runstart[runid]

    dev_c = bin_c_sorted % NCORES
    blk_c = bin_c_sorted // NCORES
    slot_c = blk_c * BCAP + pos_in_bin          # device-local candidate slot
    spos_c = seg_rank[nn_sorted] * K + j_in_seg  # block-local scatter position
    trow_c = ex[nn_sorted] * A + aug[ordc]       # teacher flat row

    seg_dev = (bin_of_seg % NCORES).astype(np.int64)
    seg_row = ((bin_of_seg // NCORES) * SEGBLK + seg_rank).astype(np.int64)

    in_maps = []
    orig_map = np.zeros((NCORES, NCAND), dtype=np.int64)
    # pad candidates scatter into the 128-row dump tail, one per partition
    pad_spos = (SEGBLK * K + np.arange(NCAND) % 128).astype(np.int32)
    for d in range(NCORES):
        m = dev_c == d
        slots = slot_c[m]
        stu_d = np.zeros((NCAND, C), dtype=np.float32)
        stu_d[slots] = stu_np[ordc[m]]
        trowf_d = np.zeros((NCAND, 1), dtype=np.int32)
        trowf_d[slots, 0] = trow_c[m]
        spos_d = pad_spos.copy()
        spos_d[slots] = spos_c[m]
        orig_map[d][slots] = ordc[m]

        # [NBLK, NT, 128] -> [NBLK, 128, NT] -> [NSEG, NT]
        def to_pt(flat):
            return np.ascontiguousarray(
                flat.reshape(NBLK, NT, 128).transpose(0, 2, 1).reshape(NSEG, NT)
            )

        cnt_d = np.zeros(NSEG, dtype=np.int32)
        soff_d = np.zeros(NSEG, dtype=np.int32)
        ms = seg_dev == np.int64(d)
        segs = np.flatnonzero(ms)
        rows = seg_row[segs]
        cnt_d[rows] = counts[segs]
        # first candidate slot of each segment on this device
        first = np.zeros(S, dtype=np.int64)
        first_mask = newseg & m
        first[nn_sorted[first_mask]] = slot_c[first_mask]
        soff_d[rows] = first[segs]

        aux_d = np.empty((NSEG, 2 * NT + 2), dtype=np.int32)
        aux_d[:, 0:NT] = to_pt(trowf_d[:, 0].copy())
        aux_d[:, NT:2 * NT] = to_pt(spos_d)
        aux_d[:, 2 * NT] = cnt_d
        aux_d[:, 2 * NT + 1] = soff_d

        in_maps.append(
            {
                "stu": stu_d,
                "aux": aux_d,
                "trowf": trowf_d,
            }
        )
    return in_maps, seg_dev, seg_row, orig_map


AMB_EPS = 1e-5  # |gap| below which the r-th pick is an fp coin-flip


def _refine_segments(segs, nn, ex, aug, stu_np, teach_np, counts, order,
                     seg_start, r):
    """Recompute the reference selection exactly (jax-CPU f32) for `segs`."""
    if len(segs) == 0:
        return {}
    cands = np.concatenate(
        [order[seg_start[s]:seg_start[s] + counts[s]] for s in segs])
    trg = ex[nn[cands]] * A + aug[cands]
    tea_rows = teach_np[trg].astype(np.float32)
    stu_rows = stu_np[cands].astype(np.float32)
    try:
        import jax
        import jax.numpy as jnp
        with jax.default_device(jax.devices("cpu")[0]):
            logq = jax.nn.log_softmax(jnp.asarray(stu_rows) / TEMP, axis=-1)
            logp = jax.nn.log_softmax(jnp.asarray(tea_rows) / TEMP, axis=-1)
            dist = np.asarray(
                jnp.sum(jnp.exp(logp) * (logp - logq), axis=-1))
    except Exception:
        xs = stu_rows / np.float32(TEMP)
        xt = tea_rows / np.float32(TEMP)
        def _ls(x):
            m = x.max(1, keepdims=True)
            e = np.exp(x - m, dtype=np.float32)
            return x - m - np.log(e.sum(1, keepdims=True, dtype=np.float32))
        logq = _ls(xs)
        logp = _ls(xt)
        dist = (np.exp(logp) * (logp - logq)).sum(1, dtype=np.float32)
    out = {}
    pos = 0
    for s in segs:
        c = counts[s]
        a = dist[pos:pos + c].astype(np.float64)
        ci = order[seg_start[s]:seg_start[s] + c]
        pos += c
        sm = -np.inf
        ag = None
        for _ in range(r):
            sm = a.max() if len(a) else -np.inf
            j = int(np.flatnonzero(a == sm)[0])  # min position = min orig idx
            ag = int(ci[j])
            a[j] = -np.inf
        out[s] = (np.float32(sm), ag)
    return out


def assemble_outputs(res, seg_dev, seg_row, orig_map, nn, ex, aug, stu_np,
                     nn_ranks_np, teach_np, r):
    m4_all = np.stack([res[d]["m4o"] for d in range(NCORES)])          # [8,NSEG,8]
    sel_all = np.stack([res[d]["selo"][:, 0] for d in range(NCORES)])  # [8,NSEG]
    tea_all = np.stack([res[d]["teao"] for d in range(NCORES)])        # [8,NSEG,C]

    mx = m4_all[seg_dev, seg_row]                                      # [S,8]
    seg_max = mx[:, r - 1]
    sel_slot = sel_all[seg_dev, seg_row].astype(np.int64)
    tea_sel = np.ascontiguousarray(tea_all[seg_dev, seg_row])

    valid = np.isfinite(seg_max) & (seg_max > -1.0e38) & (seg_max != 0)
    sel = np.where(valid, orig_map[seg_dev, sel_slot], 0).astype(np.int64)

    # ambiguity: a near-tie among real values adjacent to the cut position
    lo = max(0, r - 4)
    hi = min(7, r + 2)
    gaps = mx[:, lo:hi] - mx[:, lo + 1:hi + 1]
    both_real = (mx[:, lo:hi] > -1.0e38) & (mx[:, lo + 1:hi + 1] > -1.0e38)
    amb = ((gaps < AMB_EPS) & both_real).any(axis=1)
    segs = np.flatnonzero(amb)
    if len(segs):
        counts = np.bincount(nn, minlength=S)
        order = np.argsort(nn, kind="stable")
        seg_start = np.concatenate(([0], np.cumsum(counts)))
        rf = _refine_segments(segs, nn, ex, aug, stu_np, teach_np, counts,
                              order, seg_start, r)
        for s, (sm, ag) in rf.items():
            vs = bool(np.isfinite(sm) and sm != 0)
            valid[s] = vs
            sel[s] = ag if vs else 0
            seg_max[s] = sm
            if vs:
                tea_sel[s] = teach_np[ex[nn[ag]] * A + aug[ag]]

    nn0 = int(nn[0])
    row0 = teach_np[int(ex[nn0]) * A + int(aug[0])]
    tea_sel[~valid] = row0
    sel = np.where(valid, sel, 0)
    sel_ranks = nn_ranks_np[sel].astype(np.int32)
    return tea_sel, sel.astype(np.int32), valid.astype(bool), sel_ranks


_prog_cache = {}
last_results = None  # BassKernelResults of the most recent run (for profiling)


def kernel(augment_rank, nn_mask, example_indices, stu_logits,
           augmented_indices, nn_ranks, teacher_table):
    from concourse.bass_utils import run_bass_kernel_spmd

    r = int(augment_rank)
    assert 1 <= r <= 8
    nn_ranks_np = np.asarray(nn_ranks)
    teach_np = np.ascontiguousarray(
        np.asarray(teacher_table, dtype=np.float32).reshape(TR, C)
    )

    in_maps, seg_dev, seg_row, orig_map = prepare_inputs(
        nn_mask, example_indices, stu_logits, augmented_indices
    )
    for m in in_maps:
        m["teach"] = teach_np

    if r not in _prog_cache:
        _prog_cache[r] = build_program(r)
    nc = _prog_cache[r]

    global last_results
    import os
    trace = bool(int(os.environ.get("KERNEL_TRACE", "0")))
    last_results = run_bass_kernel_spmd(
        nc, in_maps, list(range(NCORES)), trace=trace
    )
    res = last_results.results

    return assemble_outputs(
        res, seg_dev, seg_row, orig_map,
        np.asarray(nn_mask, dtype=np.int64),
        np.asarray(example_indices, dtype=np.int64),
        np.asarray(augmented_indices, dtype=np.int64),
        np.asarray(stu_logits, dtype=np.float32),
        nn_ranks_np, teach_np, r)
